# revision 16
# baseline (speedup 1.0000x reference)
"""2D Gaussian Splatting on 8 Trainium2 NeuronCores — layout-B cumprod design.

Pixels live on partitions: each pixel-tile is 16x8 = 128 px. Per tile, the
culled gaussian list (3.5-sigma bbox, global index order) occupies a run of
free-dim columns: [spacer, g0..g_{L-1}, pads]. The 512 tiles are globally
sorted by gaussian count and dealt round-robin to the 8 cores, so every core
holds 64 tiles in 8 buckets of 8 segments; bucket heights L_j are global
maxima, making the packed geometry identical across cores (one SPMD program).

Single pass over the packed axis (chunks of 512 cols for PSUM):
  zb  = basisT @ coefs (+ logopac bias rows)      fp32r matmul K=9
  zbc = basisT @ coefs (+ (logopac+ln c) rows)    second matmul, shared rhs
        (spacer/pad cols: zb bias 0 -> alpha=1, om=0; zbc bias -60 -> AC=0)
  alpha = Exp(zb) [ACT f32]     AC = Exp(zbc) [ACT bf16]
  om = 1 - alpha  [GP tensor_scalar, bf16 out]
  s  = segmented cumprod: scan state' = max(om*state, d1p) [DVE, bf16]
       d1p built on-device: memset 0 + strided memset 1.0 at spacer cols
  wc = AC * s_shifted_one_col  [DVE bf16]  (0 at spacers/pads since AC=0)
  img column = per-bucket 3D tensor_reduce over segments  [DVE]
  out [128, 64] f32 -> host places each column as a 16x8 pixel block.
"""

import math
import numpy as np

W = 256
H = 256
TW = 16            # pixel tile width
TH = 8             # pixel tile height
NTX = W // TW      # 16
NTY = H // TH      # 32
NTILES = NTX * NTY # 512
N_CORES = 8
NT_CORE = NTILES // N_CORES   # 64 tiles per core
NSEG = 8                      # segments per bucket
NBUCK = NT_CORE // NSEG       # 8 buckets
SIGMA_K = 2.5
KQ = 9             # 5 coef rows + bias hi/lo + colorbias hi/lo
CW = 512           # chunk width (one PSUM bank of f32)
NEG = -60.0        # exp(NEG) == 0 for spacer/pad color bias


def _round_fp32r(a):
    b = np.asarray(a, np.float32).view(np.uint32).astype(np.uint64)
    r = (b + 0x7FF + ((b >> 12) & 1)) & 0xFFFFF000
    return r.astype(np.uint32).view(np.float32)


def _split_fp32r(a):
    a = np.asarray(a, np.float32)
    hi = _round_fp32r(a)
    lo = _round_fp32r(a - hi)
    return hi, lo


def _build_nc(gcap, lbs):
    """lbs: list of NBUCK bucket heights L_j (segment width is L_j + 1)."""
    import concourse.bacc as bacc
    import concourse.mybir as mybir
    from concourse.tile import TileContext

    f32 = mybir.dt.float32
    f32r = mybir.dt.float32r
    bf16 = mybir.dt.bfloat16
    AF = mybir.ActivationFunctionType
    OP = mybir.AluOpType

    chunks = [(0, min(256, gcap))]
    c0 = chunks[0][1]
    while c0 < gcap:
        cw = min(CW, gcap - c0)
        chunks.append((c0, cw))
        c0 += cw

    nc = bacc.Bacc("TRN2", target_bir_lowering=False, debug=False)
    rhs_d = nc.declare_dram_parameter("rhs", [KQ, gcap], f32r, isOutput=False)
    lhsT_d = nc.declare_dram_parameter("lhsT", [KQ, 256], f32r, isOutput=False)
    out_d = nc.declare_dram_parameter("out", [128, NT_CORE], f32, isOutput=True)

    with TileContext(nc) as tc:
        with (
            tc.tile_pool(name="const", bufs=1) as cpool,
            tc.tile_pool(name="ps", bufs=3, space="PSUM") as pspool,
        ):
            rhs_t = cpool.tile([KQ, gcap], f32r)
            lhsT_t = cpool.tile([KQ, 256], f32r)
            d1p_t = cpool.tile([128, gcap], bf16)
            abuf = cpool.tile([128, gcap], f32)
            acbuf = cpool.tile([128, gcap], bf16)
            ombuf = cpool.tile([128, gcap], bf16)
            sbufS = cpool.tile([128, gcap + 1], bf16)
            wcbuf = cpool.tile([128, gcap], bf16)
            outsb = cpool.tile([128, NT_CORE], f32)
            dummy = cpool.tile([1, 2], f32)

            # ACT exp-table preload first so the load overlaps input DMA
            nc.gpsimd.memset(dummy[:], 0.0)
            nc.scalar.activation(dummy[0:1, 0:1], dummy[0:1, 1:2], AF.Exp, bias=0.0)

            # input DMAs: rhs split so chunk-0 matmul starts early
            nc.sync.dma_start(lhsT_t[:], lhsT_d[:])
            r3 = [(0, 256), (256, 2 * CW), (2 * CW, gcap)]
            for a, b in r3:
                b = min(b, gcap)
                if b > a:
                    nc.sync.dma_start(rhs_t[:, a:b], rhs_d[:, a:b])

            # d1p built on device: zeros, then 1.0 at each bucket's spacers
            nc.gpsimd.memset(d1p_t[:], 0.0)
            off = 0
            for lb in lbs:
                seg = lb + 1
                ap3 = d1p_t[:, off : off + NSEG * seg].rearrange(
                    "p (s l) -> p s l", l=seg
                )
                nc.gpsimd.memset(ap3[:, :, 0:1], 1.0)
                off += NSEG * seg
            nc.gpsimd.memset(sbufS[:, 0:1], 0.0)

            for ci, (c0, cw) in enumerate(chunks):
                sl = slice(c0, c0 + cw)
                psA = pspool.tile([128, cw], f32, name="psA")
                psB = pspool.tile([128, cw], f32, name="psB")
                nc.tensor.matmul(
                    psA[:], lhsT_t[:, 0:128], rhs_t[:, sl], start=True, stop=True
                )
                nc.tensor.matmul(
                    psB[:], lhsT_t[:, 128:256], rhs_t[:, sl], start=True, stop=True
                )
                nc.scalar.activation(abuf[:, sl], psA[:], AF.Exp, bias=0.0)
                nc.scalar.activation(acbuf[:, sl], psB[:], AF.Exp, bias=0.0)
                nc.gpsimd.tensor_scalar(
                    ombuf[:, sl], abuf[:, sl], -1.0, 1.0, OP.mult, OP.add
                )
                nc.vector.tensor_tensor_scan(
                    sbufS[:, c0 + 1 : c0 + cw + 1],
                    ombuf[:, sl],
                    d1p_t[:, sl],
                    0.0 if ci == 0 else sbufS[:, c0 : c0 + 1],
                    OP.mult,
                    OP.max,
                )
                nc.gpsimd.tensor_mul(wcbuf[:, sl], acbuf[:, sl], sbufS[:, sl])

            off = 0
            for j, lb in enumerate(lbs):
                seg = lb + 1
                ap3 = wcbuf[:, off : off + NSEG * seg].rearrange(
                    "p (s l) -> p s l", l=seg
                )
                nc.vector.tensor_reduce(
                    outsb[:, j * NSEG : (j + 1) * NSEG],
                    ap3,
                    mybir.AxisListType.X,
                    OP.add,
                )
                nc.sync.dma_start(
                    out_d[:, j * NSEG : (j + 1) * NSEG],
                    outsb[:, j * NSEG : (j + 1) * NSEG],
                )
                off += NSEG * seg

    nc.compile()
    return nc


_NC_CACHE = {}
LAST_RESULT = None


def _get_nc(gcap, lbs):
    key = (gcap, tuple(lbs))
    if key not in _NC_CACHE:
        _NC_CACHE[key] = _build_nc(gcap, lbs)
    return _NC_CACHE[key]


def _prep_inputs(means, quats, scales, rgbs, opacities):
    """Cull + pack per core. Returns (in_maps, tile_of, gcap, lbs)."""

    means = np.asarray(means, np.float64)
    quats = np.asarray(quats, np.float64)
    scales = np.asarray(scales, np.float64)
    rgbs = np.asarray(rgbs, np.float64)
    opacities = np.asarray(opacities, np.float64)

    c = np.cos(quats)
    s = np.sin(quats)
    sx2 = scales[:, 0] ** 2
    sy2 = scales[:, 1] ** 2
    a11 = c * c * sx2 + s * s * sy2
    a12 = c * s * (sx2 - sy2)
    a22 = s * s * sx2 + c * c * sy2
    det = a11 * a22 - a12 * a12
    ia = a22 / det
    ib = -a12 / det
    ic = a11 / det
    logopac = -np.logaddexp(0.0, -opacities)
    colors = 1.0 / (1.0 + np.exp(-rgbs[:, 0]))
    lnc = np.log(colors)
    rx = SIGMA_K * np.sqrt(a11)
    ry = SIGMA_K * np.sqrt(a22)
    x0g, x1g = means[:, 0] - rx, means[:, 0] + rx
    y0g, y1g = means[:, 1] - ry, means[:, 1] + ry

    tile_idx = []
    for t in range(NTILES):
        ty, tx = divmod(t, NTX)
        X0, X1 = tx * TW, (tx + 1) * TW
        Y0, Y1 = ty * TH, (ty + 1) * TH
        idx = np.nonzero(
            (x1g >= X0) & (x0g <= X1) & (y1g >= Y0) & (y0g <= Y1)
        )[0]
        tile_idx.append(idx)

    # global sort by count desc; rank r -> core r%8, position r//8
    order = sorted(range(NTILES), key=lambda t: -len(tile_idx[t]))
    # bucket heights: max count within each rank window of 64
    lbs = [len(tile_idx[order[64 * j]]) for j in range(NBUCK)]
    gcap = sum(NSEG * (lb + 1) for lb in lbs)

    fx = (np.arange(128) % TW).astype(np.float64) - (TW - 1) / 2.0
    fy = (np.arange(128) // TW).astype(np.float64) - (TH - 1) / 2.0
    basis5 = _round_fp32r(np.stack([fx * fx, fx * fy, fy * fy, fx, fy]))
    lhsT = np.zeros((KQ, 256), np.float32)
    lhsT[0:5, 0:128] = basis5
    lhsT[5, 0:128] = 1.0
    lhsT[6, 0:128] = 1.0
    lhsT[0:5, 128:256] = basis5
    lhsT[7, 128:256] = 1.0
    lhsT[8, 128:256] = 1.0

    in_maps = []
    tile_of = np.zeros((N_CORES, NT_CORE), np.int64)
    for core in range(N_CORES):
        rhs = np.zeros((KQ, gcap), np.float32)
        rhs[7, :] = NEG  # default color-bias: exp -> 0 at spacers/pads
        col = 0
        for p in range(NT_CORE):
            j = p // NSEG
            t = order[8 * p + core]
            tile_of[core, p] = t
            idx = tile_idx[t]
            k = len(idx)
            seg = lbs[j] + 1
            base = col + 1   # after spacer
            if k:
                ty, tx = divmod(t, NTX)
                cx = tx * TW + TW / 2.0
                cy = ty * TH + TH / 2.0
                mx = means[idx, 0] - cx
                my = means[idx, 1] - cy
                iag, ibg, icg = ia[idx], ib[idx], ic[idx]
                rhs[0, base : base + k] = _round_fp32r(-0.5 * iag)
                rhs[1, base : base + k] = _round_fp32r(-ibg)
                rhs[2, base : base + k] = _round_fp32r(-0.5 * icg)
                rhs[3, base : base + k] = _round_fp32r(iag * mx + ibg * my)
                rhs[4, base : base + k] = _round_fp32r(ibg * mx + icg * my)
                bias = logopac[idx] - 0.5 * (
                    iag * mx * mx + 2 * ibg * mx * my + icg * my * my
                )
                bh, bl = _split_fp32r(bias)
                bch, bcl = _split_fp32r(bias + lnc[idx])
                rhs[5, base : base + k] = bh
                rhs[6, base : base + k] = bl
                rhs[7, base : base + k] = bch
                rhs[8, base : base + k] = bcl
            col += seg
        in_maps.append({"rhs": rhs, "lhsT": lhsT})
    return in_maps, tile_of, gcap, lbs


def _assemble(results, tile_of):
    img = np.zeros((H, W), np.float32)
    for core in range(N_CORES):
        out = np.asarray(results[core]["out"], np.float32)  # [128, NT_CORE]
        for p in range(NT_CORE):
            t = tile_of[core, p]
            ty, tx = divmod(t, NTX)
            img[ty * TH : (ty + 1) * TH, tx * TW : (tx + 1) * TW] = out[
                :, p
            ].reshape(TH, TW)
    return img[None, None].astype(np.float32)


def kernel(means, quats, scales, rgbs, opacities):
    global LAST_RESULT
    from concourse.bass_utils import run_bass_kernel_spmd

    in_maps, tile_of, gcap, lbs = _prep_inputs(means, quats, scales, rgbs, opacities)
    nc = _get_nc(gcap, lbs)
    res = run_bass_kernel_spmd(nc, in_maps, list(range(N_CORES)))
    LAST_RESULT = res
    return _assemble(res.results, tile_of)


# revision 22
# speedup vs baseline: 1.2254x; 1.2254x over previous
"""2D Gaussian Splatting on 8 Trainium2 NeuronCores — layout-B cumprod design.

Pixels live on partitions: each pixel-tile is 16x8 = 128 px. Per tile, the
culled gaussian list (3.5-sigma bbox, global index order) occupies a run of
free-dim columns: [spacer, g0..g_{L-1}, pads]. The 512 tiles are globally
sorted by gaussian count and dealt round-robin to the 8 cores, so every core
holds 64 tiles in 8 buckets of 8 segments; bucket heights L_j are global
maxima, making the packed geometry identical across cores (one SPMD program).

Single pass over the packed axis (chunks of 512 cols for PSUM):
  zb  = basisT @ coefs (+ logopac bias rows)      fp32r matmul K=9
  zbc = basisT @ coefs (+ (logopac+ln c) rows)    second matmul, shared rhs
        (spacer/pad cols: zb bias 0 -> alpha=1, om=0; zbc bias -60 -> AC=0)
  alpha = Exp(zb) [ACT f32]     AC = Exp(zbc) [ACT bf16]
  om = 1 - alpha  [GP tensor_scalar, bf16 out]
  s  = segmented cumprod: scan state' = max(om*state, d1p) [DVE, bf16]
       d1p built on-device: memset 0 + strided memset 1.0 at spacer cols
  wc = AC * s_shifted_one_col  [DVE bf16]  (0 at spacers/pads since AC=0)
  img column = per-bucket 3D tensor_reduce over segments  [DVE]
  out [128, 64] f32 -> host places each column as a 16x8 pixel block.
"""

import math
import numpy as np

W = 256
H = 256
TW = 16            # pixel tile width
TH = 8             # pixel tile height
NTX = W // TW      # 16
NTY = H // TH      # 32
NTILES = NTX * NTY # 512
N_CORES = 8
NT_CORE = NTILES // N_CORES   # 64 tiles per core
NSEG = 8                      # segments per bucket
NBUCK = NT_CORE // NSEG       # 8 buckets
SIGMA_K = 2.5
KQ = 9             # 5 coef rows + bias hi/lo + colorbias hi/lo
CW = 512           # chunk width (one PSUM bank of f32)
NEG = -60.0        # exp(NEG) == 0 for spacer/pad color bias


def _round_fp32r(a):
    b = np.asarray(a, np.float32).view(np.uint32).astype(np.uint64)
    r = (b + 0x7FF + ((b >> 12) & 1)) & 0xFFFFF000
    return r.astype(np.uint32).view(np.float32)


def _split_fp32r(a):
    a = np.asarray(a, np.float32)
    hi = _round_fp32r(a)
    lo = _round_fp32r(a - hi)
    return hi, lo


def _build_nc(gcap, lbs):
    """lbs: list of NBUCK bucket heights L_j (segment width is L_j + 1)."""
    import concourse.bacc as bacc
    import concourse.mybir as mybir
    from concourse.tile import TileContext

    f32 = mybir.dt.float32
    f32r = mybir.dt.float32r
    bf16 = mybir.dt.bfloat16
    AF = mybir.ActivationFunctionType
    OP = mybir.AluOpType

    chunks = [(0, min(256, gcap))]
    c0 = chunks[0][1]
    while c0 < gcap:
        cw = min(CW, gcap - c0)
        chunks.append((c0, cw))
        c0 += cw

    nc = bacc.Bacc("TRN2", target_bir_lowering=False, debug=False)
    inp_d = nc.declare_dram_parameter("inp", [KQ, 256 + gcap], f32r, isOutput=False)
    out_d = nc.declare_dram_parameter("out", [128, NT_CORE], f32, isOutput=True)

    with TileContext(nc) as tc:
        with (
            tc.tile_pool(name="const", bufs=1) as cpool,
            tc.tile_pool(name="ps", bufs=3, space="PSUM") as pspool,
        ):
            inp_t = cpool.tile([KQ, 256 + gcap], f32r)
            lhsT_t = inp_t[:, 0:256]
            rhs_t = inp_t[:, 256 : 256 + gcap]
            d1p_t = cpool.tile([128, gcap], bf16)
            abuf = cpool.tile([128, gcap], f32)
            acbuf = cpool.tile([128, gcap], bf16)
            ombuf = cpool.tile([128, gcap], bf16)
            sbufS = cpool.tile([128, gcap + 1], bf16)
            wcbuf = cpool.tile([128, gcap], bf16)
            outsb = cpool.tile([128, NT_CORE], f32)
            dummy = cpool.tile([1, 2], f32)

            # ACT exp-table preload first so the load overlaps input DMA
            nc.gpsimd.memset(dummy[:], 0.0)
            nc.scalar.activation(dummy[0:1, 0:1], dummy[0:1, 1:2], AF.Exp, bias=0.0)

            # input DMAs: small first piece so chunk-0 matmul starts early
            nc.sync.dma_start(inp_t[:, 0:512], inp_d[:, 0:512])
            nc.sync.dma_start(inp_t[:, 512:], inp_d[:, 512:])

            # d1p built on device: zeros, then 1.0 at each bucket's spacers
            nc.gpsimd.memset(d1p_t[:], 0.0)
            off = 0
            for lb in lbs:
                seg = lb + 1
                ap3 = d1p_t[:, off : off + NSEG * seg].rearrange(
                    "p (s l) -> p s l", l=seg
                )
                nc.gpsimd.memset(ap3[:, :, 0:1], 1.0)
                off += NSEG * seg
            nc.gpsimd.memset(sbufS[:, 0:1], 0.0)

            for ci, (c0, cw) in enumerate(chunks):
                sl = slice(c0, c0 + cw)
                psA = pspool.tile([128, cw], f32, name="psA")
                psB = pspool.tile([128, cw], f32, name="psB")
                nc.tensor.matmul(
                    psA[:], lhsT_t[:, 0:128], rhs_t[:, sl], start=True, stop=True
                )
                nc.tensor.matmul(
                    psB[:], lhsT_t[:, 128:256], rhs_t[:, sl], start=True, stop=True
                )
                nc.scalar.activation(abuf[:, sl], psA[:], AF.Exp, bias=0.0)
                nc.scalar.activation(acbuf[:, sl], psB[:], AF.Exp, bias=0.0)
                nc.gpsimd.tensor_scalar(
                    ombuf[:, sl], abuf[:, sl], -1.0, 1.0, OP.mult, OP.add
                )
                nc.vector.tensor_tensor_scan(
                    sbufS[:, c0 + 1 : c0 + cw + 1],
                    ombuf[:, sl],
                    d1p_t[:, sl],
                    0.0 if ci == 0 else sbufS[:, c0 : c0 + 1],
                    OP.mult,
                    OP.max,
                )
                nc.vector.tensor_mul(wcbuf[:, sl], acbuf[:, sl], sbufS[:, sl])

            off = 0
            for j, lb in enumerate(lbs):
                seg = lb + 1
                ap3 = wcbuf[:, off : off + NSEG * seg].rearrange(
                    "p (s l) -> p s l", l=seg
                )
                nc.vector.tensor_reduce(
                    outsb[:, j * NSEG : (j + 1) * NSEG],
                    ap3,
                    mybir.AxisListType.X,
                    OP.add,
                )
                off += NSEG * seg
            nc.sync.dma_start(out_d[:], outsb[:])

    nc.compile()
    return nc


_NC_CACHE = {}
LAST_RESULT = None


def _get_nc(gcap, lbs):
    key = (gcap, tuple(lbs))
    if key not in _NC_CACHE:
        _NC_CACHE[key] = _build_nc(gcap, lbs)
    return _NC_CACHE[key]


def _prep_inputs(means, quats, scales, rgbs, opacities):
    """Cull + pack per core. Returns (in_maps, tile_of, gcap, lbs)."""

    means = np.asarray(means, np.float64)
    quats = np.asarray(quats, np.float64)
    scales = np.asarray(scales, np.float64)
    rgbs = np.asarray(rgbs, np.float64)
    opacities = np.asarray(opacities, np.float64)

    c = np.cos(quats)
    s = np.sin(quats)
    sx2 = scales[:, 0] ** 2
    sy2 = scales[:, 1] ** 2
    a11 = c * c * sx2 + s * s * sy2
    a12 = c * s * (sx2 - sy2)
    a22 = s * s * sx2 + c * c * sy2
    det = a11 * a22 - a12 * a12
    ia = a22 / det
    ib = -a12 / det
    ic = a11 / det
    logopac = -np.logaddexp(0.0, -opacities)
    colors = 1.0 / (1.0 + np.exp(-rgbs[:, 0]))
    lnc = np.log(colors)
    rx = SIGMA_K * np.sqrt(a11)
    ry = SIGMA_K * np.sqrt(a22)
    x0g, x1g = means[:, 0] - rx, means[:, 0] + rx
    y0g, y1g = means[:, 1] - ry, means[:, 1] + ry

    tile_idx = []
    for t in range(NTILES):
        ty, tx = divmod(t, NTX)
        X0, X1 = tx * TW, (tx + 1) * TW
        Y0, Y1 = ty * TH, (ty + 1) * TH
        idx = np.nonzero(
            (x1g >= X0) & (x0g <= X1) & (y1g >= Y0) & (y0g <= Y1)
        )[0]
        tile_idx.append(idx)

    # global sort by count desc; rank r -> core r%8, position r//8
    order = sorted(range(NTILES), key=lambda t: -len(tile_idx[t]))
    # bucket heights: max count within each rank window of 64
    lbs = [len(tile_idx[order[64 * j]]) for j in range(NBUCK)]
    gcap = sum(NSEG * (lb + 1) for lb in lbs)

    fx = (np.arange(128) % TW).astype(np.float64) - (TW - 1) / 2.0
    fy = (np.arange(128) // TW).astype(np.float64) - (TH - 1) / 2.0
    basis5 = _round_fp32r(np.stack([fx * fx, fx * fy, fy * fy, fx, fy]))
    lhsT = np.zeros((KQ, 256), np.float32)
    lhsT[0:5, 0:128] = basis5
    lhsT[5, 0:128] = 1.0
    lhsT[6, 0:128] = 1.0
    lhsT[0:5, 128:256] = basis5
    lhsT[7, 128:256] = 1.0
    lhsT[8, 128:256] = 1.0

    in_maps = []
    tile_of = np.zeros((N_CORES, NT_CORE), np.int64)
    for core in range(N_CORES):
        rhs = np.zeros((KQ, gcap), np.float32)
        rhs[7, :] = NEG  # default color-bias: exp -> 0 at spacers/pads
        col = 0
        for p in range(NT_CORE):
            j = p // NSEG
            t = order[8 * p + core]
            tile_of[core, p] = t
            idx = tile_idx[t]
            k = len(idx)
            seg = lbs[j] + 1
            base = col + 1   # after spacer
            if k:
                ty, tx = divmod(t, NTX)
                cx = tx * TW + TW / 2.0
                cy = ty * TH + TH / 2.0
                mx = means[idx, 0] - cx
                my = means[idx, 1] - cy
                iag, ibg, icg = ia[idx], ib[idx], ic[idx]
                rhs[0, base : base + k] = _round_fp32r(-0.5 * iag)
                rhs[1, base : base + k] = _round_fp32r(-ibg)
                rhs[2, base : base + k] = _round_fp32r(-0.5 * icg)
                rhs[3, base : base + k] = _round_fp32r(iag * mx + ibg * my)
                rhs[4, base : base + k] = _round_fp32r(ibg * mx + icg * my)
                bias = logopac[idx] - 0.5 * (
                    iag * mx * mx + 2 * ibg * mx * my + icg * my * my
                )
                bh, bl = _split_fp32r(bias)
                bch, bcl = _split_fp32r(bias + lnc[idx])
                rhs[5, base : base + k] = bh
                rhs[6, base : base + k] = bl
                rhs[7, base : base + k] = bch
                rhs[8, base : base + k] = bcl
            col += seg
        in_maps.append({"inp": np.concatenate([lhsT, rhs], axis=1)})
    return in_maps, tile_of, gcap, lbs


def _assemble(results, tile_of):
    img = np.zeros((H, W), np.float32)
    for core in range(N_CORES):
        out = np.asarray(results[core]["out"], np.float32)  # [128, NT_CORE]
        for p in range(NT_CORE):
            t = tile_of[core, p]
            ty, tx = divmod(t, NTX)
            img[ty * TH : (ty + 1) * TH, tx * TW : (tx + 1) * TW] = out[
                :, p
            ].reshape(TH, TW)
    return img[None, None].astype(np.float32)


def kernel(means, quats, scales, rgbs, opacities):
    global LAST_RESULT
    from concourse.bass_utils import run_bass_kernel_spmd

    in_maps, tile_of, gcap, lbs = _prep_inputs(means, quats, scales, rgbs, opacities)
    nc = _get_nc(gcap, lbs)
    res = run_bass_kernel_spmd(nc, in_maps, list(range(N_CORES)))
    LAST_RESULT = res
    return _assemble(res.results, tile_of)


# revision 26
# speedup vs baseline: 1.2551x; 1.0243x over previous
"""2D Gaussian Splatting on 8 Trainium2 NeuronCores — layout-B cumprod design.

Pixels live on partitions: each pixel-tile is 16x8 = 128 px. Per tile, the
culled gaussian list (3.5-sigma bbox, global index order) occupies a run of
free-dim columns: [spacer, g0..g_{L-1}, pads]. The 512 tiles are globally
sorted by gaussian count and dealt round-robin to the 8 cores, so every core
holds 64 tiles in 8 buckets of 8 segments; bucket heights L_j are global
maxima, making the packed geometry identical across cores (one SPMD program).

Single pass over the packed axis (chunks of 512 cols for PSUM):
  zb  = basisT @ coefs (+ logopac bias rows)      fp32r matmul K=9
  zbc = basisT @ coefs (+ (logopac+ln c) rows)    second matmul, shared rhs
        (spacer/pad cols: zb bias 0 -> alpha=1, om=0; zbc bias -60 -> AC=0)
  alpha = Exp(zb) [ACT f32]     AC = Exp(zbc) [ACT bf16]
  om = 1 - alpha  [GP tensor_scalar, bf16 out]
  s  = segmented cumprod: scan state' = max(om*state, d1p) [DVE, bf16]
       d1p built on-device: memset 0 + strided memset 1.0 at spacer cols
  wc = AC * s_shifted_one_col  [DVE bf16]  (0 at spacers/pads since AC=0)
  img column = per-bucket 3D tensor_reduce over segments  [DVE]
  out [128, 64] f32 -> host places each column as a 16x8 pixel block.
"""

import math
import numpy as np

W = 256
H = 256
TW = 16            # pixel tile width
TH = 8             # pixel tile height
NTX = W // TW      # 16
NTY = H // TH      # 32
NTILES = NTX * NTY # 512
N_CORES = 8
NT_CORE = NTILES // N_CORES   # 64 tiles per core
NSEG = 8                      # segments per bucket
NBUCK = NT_CORE // NSEG       # 8 buckets
SIGMA_K = 2.25
KQ = 9             # 5 coef rows + bias hi/lo + colorbias hi/lo
CW = 512           # chunk width (one PSUM bank of f32)
NEG = -60.0        # exp(NEG) == 0 for spacer/pad color bias


def _round_fp32r(a):
    b = np.asarray(a, np.float32).view(np.uint32).astype(np.uint64)
    r = (b + 0x7FF + ((b >> 12) & 1)) & 0xFFFFF000
    return r.astype(np.uint32).view(np.float32)


def _split_fp32r(a):
    a = np.asarray(a, np.float32)
    hi = _round_fp32r(a)
    lo = _round_fp32r(a - hi)
    return hi, lo


def _build_nc(gcap, lbs):
    """lbs: list of NBUCK bucket heights L_j (segment width is L_j + 1)."""
    import concourse.bacc as bacc
    import concourse.mybir as mybir
    from concourse.tile import TileContext

    f32 = mybir.dt.float32
    f32r = mybir.dt.float32r
    bf16 = mybir.dt.bfloat16
    AF = mybir.ActivationFunctionType
    OP = mybir.AluOpType

    chunks = [(0, min(256, gcap))]
    c0 = chunks[0][1]
    while c0 < gcap:
        cw = min(CW, gcap - c0)
        chunks.append((c0, cw))
        c0 += cw

    nc = bacc.Bacc("TRN2", target_bir_lowering=False, debug=False)
    inp_d = nc.declare_dram_parameter("inp", [KQ, 256 + gcap], f32r, isOutput=False)
    out_d = nc.declare_dram_parameter("out", [128, NT_CORE], bf16, isOutput=True)

    with TileContext(nc) as tc:
        with (
            tc.tile_pool(name="const", bufs=1) as cpool,
            tc.tile_pool(name="ps", bufs=3, space="PSUM") as pspool,
        ):
            inp_t = cpool.tile([KQ, 256 + gcap], f32r)
            lhsT_t = inp_t[:, 0:256]
            rhs_t = inp_t[:, 256 : 256 + gcap]
            d1p_t = cpool.tile([128, gcap], bf16)
            abuf = cpool.tile([128, gcap], f32)
            acbuf = cpool.tile([128, gcap], bf16)
            ombuf = cpool.tile([128, gcap], bf16)
            sbufS = cpool.tile([128, gcap + 1], bf16)
            wcbuf = cpool.tile([128, gcap], bf16)
            outsb = cpool.tile([128, NT_CORE], bf16)
            dummy = cpool.tile([1, 2], f32)

            # ACT exp-table preload first so the load overlaps input DMA
            nc.gpsimd.memset(dummy[:], 0.0)
            nc.scalar.activation(dummy[0:1, 0:1], dummy[0:1, 1:2], AF.Exp, bias=0.0)

            # input DMAs: small first piece so chunk-0 matmul starts early
            nc.sync.dma_start(inp_t[:, 0:512], inp_d[:, 0:512])
            nc.sync.dma_start(inp_t[:, 512:], inp_d[:, 512:])

            # d1p built on device: zeros, then 1.0 at each bucket's spacers
            nc.gpsimd.memset(d1p_t[:], 0.0)
            off = 0
            for lb in lbs:
                seg = lb + 1
                ap3 = d1p_t[:, off : off + NSEG * seg].rearrange(
                    "p (s l) -> p s l", l=seg
                )
                nc.gpsimd.memset(ap3[:, :, 0:1], 1.0)
                off += NSEG * seg
            nc.gpsimd.memset(sbufS[:, 0:1], 0.0)

            for ci, (c0, cw) in enumerate(chunks):
                sl = slice(c0, c0 + cw)
                psA = pspool.tile([128, cw], f32, name="psA")
                psB = pspool.tile([128, cw], f32, name="psB")
                nc.tensor.matmul(
                    psA[:], lhsT_t[:, 0:128], rhs_t[:, sl], start=True, stop=True
                )
                nc.tensor.matmul(
                    psB[:], lhsT_t[:, 128:256], rhs_t[:, sl], start=True, stop=True
                )
                nc.scalar.activation(abuf[:, sl], psA[:], AF.Exp, bias=0.0)
                nc.scalar.activation(acbuf[:, sl], psB[:], AF.Exp, bias=0.0)
                nc.gpsimd.tensor_scalar(
                    ombuf[:, sl], abuf[:, sl], -1.0, 1.0, OP.mult, OP.add
                )
                nc.vector.tensor_tensor_scan(
                    sbufS[:, c0 + 1 : c0 + cw + 1],
                    ombuf[:, sl],
                    d1p_t[:, sl],
                    0.0 if ci == 0 else sbufS[:, c0 : c0 + 1],
                    OP.mult,
                    OP.max,
                )
                nc.vector.tensor_mul(wcbuf[:, sl], acbuf[:, sl], sbufS[:, sl])

            off = 0
            for j, lb in enumerate(lbs):
                seg = lb + 1
                ap3 = wcbuf[:, off : off + NSEG * seg].rearrange(
                    "p (s l) -> p s l", l=seg
                )
                with nc.allow_low_precision("image sums <=1.2; bf16 ok"):
                    nc.vector.tensor_reduce(
                        outsb[:, j * NSEG : (j + 1) * NSEG],
                        ap3,
                        mybir.AxisListType.X,
                        OP.add,
                    )
                off += NSEG * seg
            nc.sync.dma_start(out_d[:], outsb[:])

    nc.compile()
    return nc


_NC_CACHE = {}
LAST_RESULT = None


def _get_nc(gcap, lbs):
    key = (gcap, tuple(lbs))
    if key not in _NC_CACHE:
        _NC_CACHE[key] = _build_nc(gcap, lbs)
    return _NC_CACHE[key]


def _prep_inputs(means, quats, scales, rgbs, opacities):
    """Cull + pack per core. Returns (in_maps, tile_of, gcap, lbs)."""

    means = np.asarray(means, np.float64)
    quats = np.asarray(quats, np.float64)
    scales = np.asarray(scales, np.float64)
    rgbs = np.asarray(rgbs, np.float64)
    opacities = np.asarray(opacities, np.float64)

    c = np.cos(quats)
    s = np.sin(quats)
    sx2 = scales[:, 0] ** 2
    sy2 = scales[:, 1] ** 2
    a11 = c * c * sx2 + s * s * sy2
    a12 = c * s * (sx2 - sy2)
    a22 = s * s * sx2 + c * c * sy2
    det = a11 * a22 - a12 * a12
    ia = a22 / det
    ib = -a12 / det
    ic = a11 / det
    logopac = -np.logaddexp(0.0, -opacities)
    colors = 1.0 / (1.0 + np.exp(-rgbs[:, 0]))
    lnc = np.log(colors)
    rx = SIGMA_K * np.sqrt(a11)
    ry = SIGMA_K * np.sqrt(a22)
    x0g, x1g = means[:, 0] - rx, means[:, 0] + rx
    y0g, y1g = means[:, 1] - ry, means[:, 1] + ry

    tile_idx = []
    for t in range(NTILES):
        ty, tx = divmod(t, NTX)
        X0, X1 = tx * TW, (tx + 1) * TW
        Y0, Y1 = ty * TH, (ty + 1) * TH
        idx = np.nonzero(
            (x1g >= X0) & (x0g <= X1) & (y1g >= Y0) & (y0g <= Y1)
        )[0]
        tile_idx.append(idx)

    # global sort by count desc; rank r -> core r%8, position r//8
    order = sorted(range(NTILES), key=lambda t: -len(tile_idx[t]))
    # bucket heights: max count within each rank window of 64
    lbs = [len(tile_idx[order[64 * j]]) for j in range(NBUCK)]
    gcap = sum(NSEG * (lb + 1) for lb in lbs)

    fx = (np.arange(128) % TW).astype(np.float64) - (TW - 1) / 2.0
    fy = (np.arange(128) // TW).astype(np.float64) - (TH - 1) / 2.0
    basis5 = _round_fp32r(np.stack([fx * fx, fx * fy, fy * fy, fx, fy]))
    lhsT = np.zeros((KQ, 256), np.float32)
    lhsT[0:5, 0:128] = basis5
    lhsT[5, 0:128] = 1.0
    lhsT[6, 0:128] = 1.0
    lhsT[0:5, 128:256] = basis5
    lhsT[7, 128:256] = 1.0
    lhsT[8, 128:256] = 1.0

    in_maps = []
    tile_of = np.zeros((N_CORES, NT_CORE), np.int64)
    for core in range(N_CORES):
        rhs = np.zeros((KQ, gcap), np.float32)
        rhs[7, :] = NEG  # default color-bias: exp -> 0 at spacers/pads
        col = 0
        for p in range(NT_CORE):
            j = p // NSEG
            t = order[8 * p + core]
            tile_of[core, p] = t
            idx = tile_idx[t]
            k = len(idx)
            seg = lbs[j] + 1
            base = col + 1   # after spacer
            if k:
                ty, tx = divmod(t, NTX)
                cx = tx * TW + TW / 2.0
                cy = ty * TH + TH / 2.0
                mx = means[idx, 0] - cx
                my = means[idx, 1] - cy
                iag, ibg, icg = ia[idx], ib[idx], ic[idx]
                rhs[0, base : base + k] = _round_fp32r(-0.5 * iag)
                rhs[1, base : base + k] = _round_fp32r(-ibg)
                rhs[2, base : base + k] = _round_fp32r(-0.5 * icg)
                rhs[3, base : base + k] = _round_fp32r(iag * mx + ibg * my)
                rhs[4, base : base + k] = _round_fp32r(ibg * mx + icg * my)
                bias = logopac[idx] - 0.5 * (
                    iag * mx * mx + 2 * ibg * mx * my + icg * my * my
                )
                bh, bl = _split_fp32r(bias)
                bch, bcl = _split_fp32r(bias + lnc[idx])
                rhs[5, base : base + k] = bh
                rhs[6, base : base + k] = bl
                rhs[7, base : base + k] = bch
                rhs[8, base : base + k] = bcl
            col += seg
        in_maps.append({"inp": np.concatenate([lhsT, rhs], axis=1)})
    return in_maps, tile_of, gcap, lbs


def _assemble(results, tile_of):
    img = np.zeros((H, W), np.float32)
    for core in range(N_CORES):
        out = np.asarray(results[core]["out"], np.float32)  # [128, NT_CORE]
        for p in range(NT_CORE):
            t = tile_of[core, p]
            ty, tx = divmod(t, NTX)
            img[ty * TH : (ty + 1) * TH, tx * TW : (tx + 1) * TW] = out[
                :, p
            ].reshape(TH, TW)
    return img[None, None].astype(np.float32)


def kernel(means, quats, scales, rgbs, opacities):
    global LAST_RESULT
    from concourse.bass_utils import run_bass_kernel_spmd

    in_maps, tile_of, gcap, lbs = _prep_inputs(means, quats, scales, rgbs, opacities)
    nc = _get_nc(gcap, lbs)
    res = run_bass_kernel_spmd(nc, in_maps, list(range(N_CORES)))
    LAST_RESULT = res
    return _assemble(res.results, tile_of)
